# revision 47
# baseline (speedup 1.0000x reference)
"""GQA attention (RoPE + causal softmax + out-proj) on 8 TRN2 NeuronCores.

Problem (hardcoded): B=2, S=2048, D=1024, H=16 heads, 4 KV heads, head_dim 64.

Sharding: core c -> batch b = c//4, head-group r = c%4 (4 query heads, KV head
r -- GQA groups align exactly).  Every core runs an IDENTICAL program (SPMD);
all per-core variation lives in the input data and in partition_id-derived
DRAM offsets.

Per-core pipeline:
  1. xv first (seq-quarter-sharded) so its 8-core AllToAll issues as early
     as possible -- slot j carries only kv head j%4's [512,64] block (the
     DRAM round-trip doubles as the [k, f]-orientation transpose;
     cross-batch slots are ignored by the receiver).  xk and xq follow,
     computed locally (own KV head / own 4 query heads, full sequence) and
     hide the exchange.  Odd query heads hop to partition-base-0 tiles via
     SBUF DMA (PE matmuls with base-64 operands hang this stack).
  2. Attention in k-on-partition layout, 512-query blocks: scoresT chunks
     [128k, 512q] x 2 heads -> one [128,1024] exp on ScalarE per key chunk
     (scale=1/8 folded; PSUM sources must stay <= 4KB/partition); causal =
     chunk skipping + mask muls on the 4 diagonal chunks; attn@v with
     lhsT=[v | 64 ones-cols] (M=128, same cycles as M=65) accumulating
     [128, 512q] per head in one PSUM bank -- partitions 64..127 receive
     the softmax denominator already replicated, so normalize is a
     partition-aligned DVE reciprocal + multiply.  N=512 matmuls
     throughout: per-instruction overhead, not FLOPs, binds the PE here.
  3. Attention outputs exchanged per head-pair via 8-core AllToAll (slot j
     = the [128 f, 512 seq] quarter core j needs; 4x less wire traffic
     than an AllGather; pair 0's exchange overlaps pair 1's compute);
     out-projection holds all 8 dt tiles in PSUM and accumulates the
     pair-0 f-chunks while pair 1's exchange is in flight; bf16 output.

kernel(**inputs) accepts the FULL unsharded inputs and returns [2,2048,1024].
"""

import numpy as np
import ml_dtypes

B, S, D = 2, 2048, 1024
H, HKV, DH = 16, 4, 64
SCALE = 1.0 / 8.0
NCORES = 8
Q4 = 512  # seq quarter per core (kv projections)
QB = 256  # attention query block
NQB = S // QB
BF = ml_dtypes.bfloat16

VSLOT = 512 * 64  # v AllToAll slot: [512 seq-quarter k, one kv head's 64 f]
PSLOT = 128 * 512  # attention-out AllToAll slot: [2 heads x 64 f, 512 seq]

_CACHE = None


def _build(reps=1, ablate=(), opts=()):
    """Build the SPMD program.  reps=1 is the production kernel; reps>1
    emits the ENTIRE per-core pipeline (input DMAs included) back-to-back
    `reps` times with per-rep collective DRAM buffers -- used only by
    bench() to measure the marginal per-execution device time, since the
    axon tunnel's ~75ms blocking-dispatch floor swamps a single run.
    `ablate` (perf-experiment only, never used for the graded kernel):
    subset of {"agv","aga"} -- skip those collectives (numerics break)."""
    import concourse.bass as bass
    import concourse.bacc as bacc
    import concourse.mybir as mybir
    import concourse.tile as tile

    F32 = mybir.dt.float32
    BF16 = mybir.dt.bfloat16
    EXP = mybir.ActivationFunctionType.Exp

    nc = bacc.Bacc("TRN2", target_bir_lowering=False, debug=False, num_devices=NCORES)

    qT_e = nc.dram_tensor("qT", [D, S], BF16, kind="ExternalInput")
    kT_e = nc.dram_tensor("kT", [D, S], BF16, kind="ExternalInput")
    vT_e = nc.dram_tensor("vT", [D, Q4], BF16, kind="ExternalInput")
    wqT_e = nc.dram_tensor("wqT", [D, 256], BF16, kind="ExternalInput")
    wkvT_e = nc.dram_tensor("wkvT", [D, 320], BF16, kind="ExternalInput")
    woT_e = nc.dram_tensor("woT", [D, 1024], BF16, kind="ExternalInput")
    cq_e = nc.dram_tensor("cq", [128, S], BF16, kind="ExternalInput")
    sq_e = nc.dram_tensor("sq", [128, S], BF16, kind="ExternalInput")
    out_e = nc.dram_tensor("out", [1024, Q4], BF16, kind="ExternalOutput")

    groups8 = [list(range(NCORES))]
    PAIRSWAP = [i ^ 1 for i in range(32)]

    with tile.TileContext(nc) as tc:
        with (
            tc.tile_pool(name="sb", bufs=1) as sb,
            tc.tile_pool(name="dram", bufs=1, space="DRAM") as dp,
        ):
            pid = nc.sync.partition_id()
            r_sv = nc.sync.snap(pid % 4)
            kvv_base = nc.sync.snap((pid // 4) * (4 * VSLOT))
            a_base = nc.sync.snap((pid // 4) * (4 * PSLOT))
            a_base2 = nc.sync.snap((pid // 4) * (8 * PSLOT))

            for rep in range(reps):
                _emit_rep(
                    nc, tc, sb, dp, bass, mybir, tile, F32, BF16, EXP,
                    qT_e, kT_e, vT_e, wqT_e, wkvT_e, woT_e, cq_e, sq_e, out_e,
                    groups8, PAIRSWAP, rep,
                    r_sv, kvv_base, a_base, a_base2, ablate, opts,
                )

    nc.compile()
    return nc


def _emit_rep(
    nc, tc, sb, dp, bass, mybir, tile, F32, BF16, EXP,
    qT_e, kT_e, vT_e, wqT_e, wkvT_e, woT_e, cq_e, sq_e, out_e,
    groups8, PAIRSWAP, rep,
    r_sv, kvv_base, a_base, a_base2=None, ablate=(), opts=(),
):
    if True:  # keep the original body's indentation
        if True:
            # v exchange: 8-core AllToAll, slot j = kv head j%4's [512, 64]
            # block of my seq quarter (cross-batch slots ignored by receiver)
            agv_in = dp.tile([NCORES * VSLOT], BF16, name=f"agv_in_r{rep}")
            agv_out = dp.tile([NCORES * VSLOT], BF16, name=f"agv_out_r{rep}")
            # attention-out exchange: 8-core AllToAll, slot j = my pair's
            # [128 f, 512 seq] quarter that core j needs (cross-batch slots
            # carry duplicate data the receiver ignores) -- 4x less wire
            # traffic than the AllGather of the full [128 f, 2048 seq].
            if "aga1x" in opts:
                aga_in = [dp.tile([NCORES * 2 * PSLOT], BF16,
                                  name=f"aga_in_r{rep}")]
                aga_out = [dp.tile([NCORES * 2 * PSLOT], BF16,
                                   name=f"aga_out_r{rep}")]
            else:
                aga_in = [
                    dp.tile([NCORES * PSLOT], BF16, name=f"aga_in{i}_r{rep}")
                    for i in range(2)
                ]
                aga_out = [
                    dp.tile([NCORES * PSLOT], BF16, name=f"aga_out{i}_r{rep}")
                    for i in range(2)
                ]

            # prime the exp table set early (~2.7us load) with a dep-free input
            dummy_in = sb.tile([1, 8], F32, name="dummy_in")
            nc.vector.memset(dummy_in[:], 0.25)
            dummy = sb.tile([1, 8], F32, name="dummy")
            nc.scalar.activation(dummy[:], dummy_in[:], EXP, scale=0.001)

            # mask variant d (cols [512d, 512d+512)): diagonal key-chunk d
            # of a 512-query block; keep where q >= k + 128d
            mdiag = sb.tile([128, 2048], BF16, name="mdiag")
            nc.vector.memset(mdiag[:], 1.0)
            for d in range(4):
                sl = slice(512 * d, 512 * (d + 1))
                nc.gpsimd.affine_select(
                    out=mdiag[:, sl],
                    in_=mdiag[:, sl],
                    compare_op=mybir.AluOpType.is_ge,
                    fill=0.0,
                    base=-128 * d,
                    pattern=[[1, 512]],
                    channel_multiplier=-1,
                )

            # ---------------- phase 1: k/v projections + AG ----------------
            kts = [sb.tile([128, S], BF16, name=f"kts{i}") for i in range(8)]
            vts = [sb.tile([128, Q4], BF16, name=f"vts{i}") for i in range(8)]
            wkv = [sb.tile([128, 320], BF16, name=f"wkv{i}") for i in range(8)]
            for i in range(8):
                sl = slice(128 * i, 128 * (i + 1))
                # v + weights first: the v exchange is the first collective
                # and everything downstream of it benefits from an early start
                nc.sync.dma_start(out=vts[i][:], in_=vT_e.ap()[sl, :])
                nc.sync.dma_start(out=wkv[i][:], in_=wkvT_e.ap()[sl, :])
            for i in range(8):
                sl = slice(128 * i, 128 * (i + 1))
                nc.sync.dma_start(out=kts[i][:], in_=kT_e.ap()[sl, :])

            # rope tables (row pattern has period 64, so rows 0:64 serve the
            # single local KV head too; k and q positions are both 0..S)
            cq = sb.tile([128, S], BF16, name="cq")
            sq = sb.tile([128, S], BF16, name="sq")
            for t_, e_ in ((cq, cq_e), (sq, sq_e)):
                nc.sync.dma_start(out=t_[:], in_=e_.ap())

            xkg = sb.tile([64, S], BF16, name="xkg")
            with tc.tile_pool(name="ppp", bufs=3, space="PSUM") as ppp:
                for kt in range(4):  # xv: [512 k, 256 f] -> 4 tiles
                    pv = ppp.tile([128, 256], F32, name="pv", tag="proj")
                    for dc in range(8):
                        nc.tensor.matmul(
                            pv[:],
                            vts[dc][:, kt * 128 : (kt + 1) * 128],
                            wkv[dc][:, 64:320],
                            start=(dc == 0),
                            stop=(dc == 7),
                        )
                    xvb = sb.tile([128, 256], BF16, name="xvb", bufs=2)
                    nc.vector.tensor_copy(xvb[:], pv[:])
                    # write only the same-batch slot group (cross-batch slots
                    # are ignored by their receivers, so they stay stale):
                    # slot j=4b+r' rows [128kt,+128) col block r' -- one DMA
                    vsl = agv_in[bass.ds(kvv_base, 4 * VSLOT)].rearrange(
                        "(j p f) -> p j f", j=4, p=Q4, f=64
                    )
                    nc.sync.dma_start(
                        out=vsl[128 * kt : 128 * (kt + 1), :, :], in_=xvb[:]
                    )

                if "agv" not in ablate:
                    nc.gpsimd.collective_compute(
                        "AllToAll",
                        mybir.AluOpType.bypass,
                        replica_groups=groups8,
                        ins=[agv_in[:].opt()],
                        outs=[agv_out[:].opt()],
                    )

                # xk for the core's own KV head over the full sequence --
                # no collective on the scores-critical path
                for kc in range(4):
                    ksl = slice(512 * kc, 512 * (kc + 1))
                    pk = ppp.tile([64, Q4], F32, name="pk64", tag="proj")
                    for dc in range(8):
                        nc.tensor.matmul(
                            pk[:],
                            wkv[dc][:, 0:64],
                            kts[dc][:, ksl],
                            start=(dc == 0),
                            stop=(dc == 7),
                        )
                    xsw = sb.tile([64, Q4], F32, name="xswk", bufs=2)
                    t1 = sb.tile([64, Q4], F32, name="t1k", bufs=2)
                    t2 = sb.tile([64, Q4], F32, name="t2k", bufs=2)
                    nc.vector.stream_shuffle(xsw[:], pk[:], PAIRSWAP)
                    nc.vector.tensor_mul(t1[:], pk[:], cq[0:64, ksl])
                    nc.vector.tensor_mul(t2[:], xsw[:], sq[0:64, ksl])
                    nc.vector.tensor_add(xkg[:, ksl], t1[:], t2[:])

                # -------- phase 2: local xq projection (own 4 heads, full S)
                qts = [sb.tile([128, S], BF16, name=f"qts{i}") for i in range(8)]
                wqs = [sb.tile([128, 256], BF16, name=f"wqs{i}") for i in range(8)]
                for i in range(8):
                    sl = slice(128 * i, 128 * (i + 1))
                    nc.sync.dma_start(out=qts[i][:], in_=qT_e.ap()[sl, :])
                    nc.sync.dma_start(out=wqs[i][:], in_=wqT_e.ap()[sl, :])

                xqr = [sb.tile([128, S], BF16, name=f"xqr{t}") for t in range(2)]
                xqodd = [sb.tile([64, S], BF16, name=f"xqodd{t}") for t in range(2)]
                for t in range(2):
                    for qc in range(4):
                        qsl = slice(512 * qc, 512 * (qc + 1))
                        pq = ppp.tile([128, Q4], F32, name="pk", tag="proj")
                        for dc in range(8):
                            nc.tensor.matmul(
                                pq[:],
                                wqs[dc][:, t * 128 : (t + 1) * 128],
                                qts[dc][:, qsl],
                                start=(dc == 0),
                                stop=(dc == 7),
                            )
                        xsw = sb.tile([128, Q4], F32, name="xsw", bufs=2)
                        t1 = sb.tile([128, Q4], F32, name="t1", bufs=2)
                        t2 = sb.tile([128, Q4], F32, name="t2", bufs=2)
                        nc.vector.stream_shuffle(xsw[:], pq[:], PAIRSWAP)
                        nc.vector.tensor_mul(t1[:], pq[:], cq[:, qsl])
                        nc.vector.tensor_mul(t2[:], xsw[:], sq[:, qsl])
                        nc.vector.tensor_add(xqr[t][:, qsl], t1[:], t2[:])
                        # odd heads hop to base-0 per chunk, so their scores
                        # start as soon as each rope chunk lands
                        nc.sync.dma_start(
                            out=xqodd[t][:, qsl], in_=xqr[t][64:128, qsl]
                        )

            # ---------------- phase 3: v assembly from the AllToAll --------
            # vaug: [128, 16*128], chunk c cols [128c, 128c+64) = v rows,
            # cols [128c+64, 128c+128) = 1.0: the attn@v matmul (M=128, same
            # cycles as M=65) then lands the softmax denominator replicated
            # on partitions 64..127, so normalize needs no partition moves.
            # chunk c = rows [128(c%4), +128) of same-batch peer (c//4)'s slot
            # same-batch slots are contiguous: flat addr = kvv_base + c*8192
            # + p*64 + col for chunk c = 0..15 -> one strided DMA fills all
            # 16 v column-blocks; the ones-columns are memset per chunk
            vaug = sb.tile([128, 16 * 128], BF16, name="vaug")
            vsrc = agv_out[bass.ds(kvv_base, 4 * VSLOT)].rearrange(
                "(c p f) -> p c f", c=16, p=128, f=64
            )
            vdst = vaug[:].rearrange("p (c f) -> p c f", c=16, f=128)[:, :, 0:64]
            nc.sync.dma_start(out=vdst, in_=vsrc)
            for c in range(16):
                nc.vector.memset(vaug[:, 128 * c + 64 : 128 * (c + 1)], 1.0)

            # prefetch wo weights (after the attn-critical vaug/xq DMAs in
            # priority order; only needed at the very end)
            wos = [sb.tile([128, 1024], BF16, name=f"wos{i}") for i in range(8)]
            for i in range(8):
                nc.sync.dma_start(
                    out=wos[i][:], in_=woT_e.ap()[128 * i : 128 * (i + 1), :]
                )

            # ---------------- phase 4: attention ----------------
            stage = [sb.tile([64, S], BF16, name=f"stage{h}") for h in range(4)]
            if "exp" in ablate:
                etconst = sb.tile([128, 1024], BF16, name="etconst")
                nc.vector.memset(etconst[:], 0.01)
            with (
                tc.tile_pool(
                    name="psc", bufs=(3 if "psum2" in opts else 2), space="PSUM"
                ) as psc,
                tc.tile_pool(
                    name="pacc", bufs=(1 if "psum2" in opts else 2), space="PSUM"
                ) as pacc,
            ):
                for p in range(2):  # head pair (local heads 2p, 2p+1)
                    for qb in range(4):  # 512-query blocks
                        qo = 512 * qb
                        nch = 4 * qb + 4
                        acc = [
                            pacc.tile([128, 512], F32, name=f"acc{half}")
                            for half in range(2)
                        ]
                        for c in range(nch):  # one 128-key chunk per group
                            scp = psc.tile([128, 1024], F32, name="scp")
                            ko = 128 * c
                            d = c - (nch - 4)
                            # diagonal chunk d only reaches queries q >= 128d
                            # (q < 128d is fully masked): trim scores, exp,
                            # mask, and attn@v to that column range
                            qlo = 128 * d if d > 0 else 0
                            for half in range(2):
                                h = 2 * p + half
                                rhs = (
                                    xqr[p][0:64, qo + qlo : qo + 512]
                                    if half == 0
                                    else xqodd[p][:, qo + qlo : qo + 512]
                                )
                                nc.tensor.matmul(
                                    scp[:, 512 * half + qlo : 512 * (half + 1)],
                                    xkg[:, ko : ko + 128],
                                    rhs,
                                    start=True,
                                    stop=True,
                                )
                            et = sb.tile([128, 1024], BF16, name="et", bufs=3)
                            if "exp" in ablate:
                                # perf probe: release scp via a tiny ACT read
                                # and feed attn@v a constant (numerics break)
                                nc.scalar.activation(
                                    et[0:1, 0:8], scp[0:1, 0:8], EXP, scale=SCALE
                                )
                                et = etconst
                            else:
                                if qlo == 0 and "exp2" not in opts:
                                    nc.scalar.activation(
                                        et[:], scp[:], EXP, scale=SCALE
                                    )
                                else:
                                    for half in range(2):
                                        esl = slice(
                                            512 * half + qlo, 512 * (half + 1)
                                        )
                                        nc.scalar.activation(
                                            et[:, esl], scp[:, esl],
                                            EXP, scale=SCALE,
                                        )
                            if d >= 0 and "exp" not in ablate:
                                for half in range(2):
                                    esl = slice(512 * half + qlo, 512 * (half + 1))
                                    nc.vector.tensor_mul(
                                        et[:, esl], et[:, esl],
                                        mdiag[:, 512 * d + qlo : 512 * (d + 1)],
                                    )
                            for half in range(2):
                                nc.tensor.matmul(
                                    acc[half][:, qlo:512],
                                    vaug[:, 128 * c : 128 * (c + 1)],
                                    et[:, 512 * half + qlo : 512 * (half + 1)],
                                    start=(c == 0),
                                    stop=(c == nch - 1),
                                )
                        rec = sb.tile([64, 1024], F32, name="rec", bufs=2)
                        for half in range(2):
                            h = 2 * p + half
                            rsl = slice(half * 512, (half + 1) * 512)
                            nc.vector.reciprocal(rec[:, rsl], acc[half][64:128, :])
                            nc.vector.tensor_mul(
                                stage[h][:, qo : qo + 512],
                                acc[half][0:64, :],
                                rec[:, rsl],
                            )
                    # ship this pair's attention output to the same-batch
                    # slot group (slot 4b+r' cols = seq quarter r'): the
                    # rearranged view scatters stage's 4 quarters across
                    # slots in one DMA per head
                    if "aga1x" in opts:
                        aview = aga_in[0][
                            bass.ds(a_base2, 8 * PSLOT)
                        ].rearrange("(j p f) -> p j f", j=4, p=256, f=Q4)
                        for half in range(2):
                            h = 2 * p + half
                            nc.sync.dma_start(
                                out=aview[h * 64 : (h + 1) * 64, :, :],
                                in_=stage[h][:],
                            )
                        if p == 1 and "aga" not in ablate:
                            nc.gpsimd.collective_compute(
                                "AllToAll",
                                mybir.AluOpType.bypass,
                                replica_groups=groups8,
                                ins=[aga_in[0][:].opt()],
                                outs=[aga_out[0][:].opt()],
                            )
                        continue
                    aview = aga_in[p][bass.ds(a_base, 4 * PSLOT)].rearrange(
                        "(j p f) -> p j f", j=4, p=128, f=Q4
                    )
                    for half in range(2):
                        h = 2 * p + half
                        nc.sync.dma_start(
                            out=aview[half * 64 : (half + 1) * 64, :, :],
                            in_=stage[h][:],
                        )
                    if "aga" not in ablate:
                        nc.gpsimd.collective_compute(
                            "AllToAll",
                            mybir.AluOpType.bypass,
                            replica_groups=groups8,
                            ins=[aga_in[p][:].opt()],
                            outs=[aga_out[p][:].opt()],
                        )

            # ---------------- phase 5: out-projection ----------------
            fcs = [2 * g for g in range(4)] + [2 * g + 1 for g in range(4)]
            if "aga1x" in opts:
                # slot g = peer g's 4 heads: f-chunk u at flat a_base2 +
                # u*PSLOT (contiguous) -> one readback DMA for all 8 chunks
                wora = sb.tile([128, 8 * Q4], BF16, name="wora")
                view = aga_out[0][
                    bass.ds(a_base2, 8 * PSLOT)
                ].rearrange("(u p f) -> p u f", u=8, p=128, f=Q4)
                nc.sync.dma_start(out=wora[:], in_=view)

                def worhs_ap(u):
                    return wora[:, Q4 * u : Q4 * (u + 1)]
            else:
                # wo_rhs tile u (f rows [128u, 128u+128)): u = 2g+par = slot
                # (4b+g) of aga_out[par]; the 4 same-batch slots are
                # contiguous -> one [128, 4*512] readback DMA per parity
                wor = [
                    sb.tile([128, 4 * Q4], BF16, name=f"wor{par}")
                    for par in range(2)
                ]
                for par in range(2):
                    view = aga_out[par][
                        bass.ds(a_base, 4 * PSLOT)
                    ].rearrange("(g p f) -> p g f", g=4, p=128, f=Q4)
                    nc.sync.dma_start(out=wor[par][:], in_=view)

                def worhs_ap(u):
                    g, par = divmod(u, 2)
                    return wor[par][:, Q4 * g : Q4 * (g + 1)]

            # all 8 dt tiles live in PSUM at once (one bank each): the even
            # f-chunk accumulations depend only on the pair-0 exchange, so
            # they all run while the pair-1 exchange is still in flight;
            # odds + copyback follow when aga_out[1] lands
            if "wo_serial" in opts or "aga1x" in opts:
                with tc.tile_pool(name="pwo", bufs=3, space="PSUM") as pwo:
                    for dt in range(8):
                        wop = pwo.tile([128, Q4], F32, name="wop")
                        for i, fc in enumerate(fcs):
                            nc.tensor.matmul(
                                wop[:],
                                wos[fc][:, dt * 128 : (dt + 1) * 128],
                                worhs_ap(fc),
                                start=(i == 0),
                                stop=(i == 7),
                            )
                        ob = sb.tile([128, Q4], BF16, name="ob", bufs=4)
                        if dt % 2 == 0:
                            nc.vector.tensor_copy(ob[:], wop[:])
                        else:
                            nc.scalar.copy(ob[:], wop[:])
                        nc.sync.dma_start(
                            out=out_e.ap()[128 * dt : 128 * (dt + 1), :], in_=ob[:]
                        )
                return
            # all 8 dt tiles live in PSUM at once (one bank each): the even
            # f-chunk accumulations depend only on the pair-0 exchange, so
            # they all run while the pair-1 exchange is still in flight;
            # odds + copyback follow when aga_out[1] lands
            with tc.tile_pool(name="pwo", bufs=1, space="PSUM") as pwo:
                wop = [pwo.tile([128, Q4], F32, name=f"wop{dt}") for dt in range(8)]
                for dt in range(8):
                    for i, fc in enumerate(fcs[:4]):  # evens (pair-0 data)
                        nc.tensor.matmul(
                            wop[dt][:],
                            wos[fc][:, dt * 128 : (dt + 1) * 128],
                            worhs_ap(fc),
                            start=(i == 0),
                            stop=False,
                        )
                for dt in range(8):
                    for i, fc in enumerate(fcs[4:]):  # odds (pair-1 data)
                        nc.tensor.matmul(
                            wop[dt][:],
                            wos[fc][:, dt * 128 : (dt + 1) * 128],
                            worhs_ap(fc),
                            start=False,
                            stop=(i == 3),
                        )
                    ob = sb.tile([128, Q4], BF16, name="ob", bufs=4)
                    # split the PSUM copyback across DVE and ACT (both idle
                    # in the tail) so the out DMAs start sooner
                    if dt % 2 == 0:
                        nc.vector.tensor_copy(ob[:], wop[dt][:])
                    else:
                        nc.scalar.copy(ob[:], wop[dt][:])
                    nc.sync.dma_start(
                        out=out_e.ap()[128 * dt : 128 * (dt + 1), :], in_=ob[:]
                    )


_RUNNERS = {}


def _get_runner(nc):
    """Cached jitted shard_map executor (mirrors bass2jax.run_bass_via_pjrt's
    multi-core branch, but compiled once so repeat calls just execute)."""
    if id(nc) in _RUNNERS:
        return _RUNNERS[id(nc)]
    import jax
    import numpy as _np
    import concourse.mybir as mybir
    from concourse import bass2jax
    from jax.sharding import Mesh, PartitionSpec
    from jax.experimental.shard_map import shard_map

    bass2jax.install_neuronx_cc_hook()

    partition_name = nc.partition_id_tensor.name if nc.partition_id_tensor else None
    in_names, out_names, out_avals, zero_shapes = [], [], [], []
    for alloc in nc.m.functions[0].allocations:
        if not isinstance(alloc, mybir.MemoryLocationSet):
            continue
        name = alloc.memorylocations[0].name
        if alloc.kind == "ExternalInput":
            if name != partition_name:
                in_names.append(name)
        elif alloc.kind == "ExternalOutput":
            out_avals.append(
                jax.core.ShapedArray(tuple(alloc.tensor_shape), mybir.dt.np(alloc.dtype))
            )
            out_names.append(name)
            zero_shapes.append((tuple(alloc.tensor_shape), mybir.dt.np(alloc.dtype)))

    n_params = len(in_names)
    all_in_names = list(in_names) + list(out_names)
    if partition_name is not None:
        all_in_names.append(partition_name)

    def _body(*args):
        operands = list(args)
        if partition_name is not None:
            operands.append(bass2jax.partition_id_tensor())
        outs = bass2jax._bass_exec_p.bind(
            *operands,
            out_avals=tuple(out_avals),
            in_names=tuple(all_in_names),
            out_names=tuple(out_names),
            lowering_input_output_aliases=(),
            sim_require_finite=True,
            sim_require_nnan=True,
            nc=nc,
        )
        return tuple(outs)

    devices = jax.devices()[:NCORES]
    mesh = Mesh(_np.asarray(devices), ("core",))
    in_specs = (PartitionSpec("core"),) * (n_params + len(out_names))
    out_specs = (PartitionSpec("core"),) * len(out_names)
    sharded = jax.jit(
        shard_map(_body, mesh=mesh, in_specs=in_specs, out_specs=out_specs, check_rep=False),
        keep_unused=True,
    )
    sharding = jax.sharding.NamedSharding(mesh, PartitionSpec("core"))

    def to_device(in_maps):
        per_core = [[np.asarray(m[name]) for name in in_names] for m in in_maps]
        concat_in = [
            np.concatenate([per_core[c][i] for c in range(NCORES)], axis=0)
            for i in range(n_params)
        ]
        concat_in += [
            np.zeros((NCORES * shp[0], *shp[1:]), dt) for shp, dt in zero_shapes
        ]
        return [jax.device_put(a, sharding) for a in concat_in]

    def execute(dev_args):
        out_arrs = sharded(*dev_args)
        jax.block_until_ready(out_arrs)
        return out_arrs

    def run(in_maps):
        out_arrs = execute(to_device(in_maps))
        return [
            {
                name: np.asarray(out_arrs[i]).reshape(NCORES, *out_avals[i].shape)[c]
                for i, name in enumerate(out_names)
            }
            for c in range(NCORES)
        ]

    run.to_device = to_device
    run.execute = execute
    run.sharded = sharded
    _RUNNERS[id(nc)] = run
    return run


def make_in_maps(query, key, value, freqs_cos, freqs_sin, wq, wk, wv, wo):
    query = np.asarray(query, dtype=np.float32)
    key = np.asarray(key, dtype=np.float32)
    value = np.asarray(value, dtype=np.float32)
    freqs_cos = np.asarray(freqs_cos, dtype=np.float32)
    freqs_sin = np.asarray(freqs_sin, dtype=np.float32)

    wqT = np.ascontiguousarray(np.asarray(wq, np.float32).T).astype(BF)  # [D, 1024]
    wkT = np.ascontiguousarray(np.asarray(wk, np.float32).T).astype(BF)  # [D, 256]
    wvT = np.ascontiguousarray(np.asarray(wv, np.float32).T).astype(BF)  # [D, 256]
    woT = np.ascontiguousarray(np.asarray(wo, np.float32).T).astype(BF)

    p = np.arange(128)
    j = (p % 64) // 2
    sign = np.where(p % 2 == 0, -1.0, 1.0).astype(np.float32)

    cq_full = np.ascontiguousarray(freqs_cos[:, j].T).astype(BF)  # [128, S]
    sq_full = np.ascontiguousarray(freqs_sin[:, j].T * sign[:, None]).astype(BF)

    qT_full = [
        np.ascontiguousarray(query[b].T).astype(BF) for b in range(B)
    ]  # [D, S] each
    kT_full = [np.ascontiguousarray(key[b].T).astype(BF) for b in range(B)]

    in_maps = []
    for c in range(NCORES):
        b, r = divmod(c, 4)
        rows = slice(Q4 * r, Q4 * (r + 1))
        vT = np.ascontiguousarray(value[b, rows, :].T).astype(BF)
        # wkvT: cols 0:64 = wk rows of my KV head (transposed), 64:320 = wv.T
        wkvT = np.ascontiguousarray(
            np.concatenate([wkT[:, 64 * r : 64 * (r + 1)], wvT], axis=1)
        )
        in_maps.append(
            {
                "qT": qT_full[b],
                "kT": kT_full[b],
                "vT": vT,
                "wqT": np.ascontiguousarray(wqT[:, 256 * r : 256 * (r + 1)]),
                "wkvT": wkvT,
                "woT": woT,
                "cq": cq_full,
                "sq": sq_full,
            }
        )
    return in_maps


def kernel(query, key, value, freqs_cos, freqs_sin, wq, wk, wv, wo):
    global _CACHE
    from concourse.bass_utils import run_bass_kernel_spmd

    if _CACHE is None:
        _CACHE = _build()
    nc = _CACHE

    in_maps = make_in_maps(query, key, value, freqs_cos, freqs_sin, wq, wk, wv, wo)
    results = run_bass_kernel_spmd(nc, in_maps, list(range(NCORES))).results
    LAST_IN_MAPS[:] = in_maps

    out = np.empty((B, S, D), np.float32)
    for c in range(NCORES):
        b, r = divmod(c, 4)
        out[b, Q4 * r : Q4 * (r + 1), :] = results[c]["out"].T
    return out


LAST_IN_MAPS = []


BENCH_REPS_LO = 33
BENCH_REPS_HI = 65


def _launch_block(run, dev, n_launches):
    """Pipeline n_launches async executes, block once; per-launch seconds."""
    import time
    import jax

    t0 = time.perf_counter()
    outs = None
    for _ in range(n_launches):
        outs = run.sharded(*dev)
    jax.block_until_ready(outs)
    return (time.perf_counter() - t0) / n_launches


_CACHE_REP = {}


def bench(n=10, reps_lo=BENCH_REPS_LO, reps_hi=BENCH_REPS_HI,
          n_launches=16, trials=11, rounds=3, settle_s=10.0,
          opts=(), ablate=()):
    """Measure the per-execution device time of the kernel.

    The axon tunnel's blocking-dispatch floor is ~75ms with +/-10ms jitter
    and a per-launch streaming cost of ~2-3ms, so neither a blocking
    execute() nor a single-kernel launch can resolve the ~0.2ms device
    time.  Instead we compile two programs that run the FULL per-core
    pipeline (input DMAs, projections, collectives, attention,
    out-projection) reps_lo and reps_hi times back-to-back with per-rep
    collective buffers.  Both launches are device-bound (device time >>
    per-launch streaming cost), so in the pipelined regime their wall
    times differ by exactly the device time of (reps_hi - reps_lo)
    kernel executions:

        t_hw = (T_hi - T_lo) / (reps_hi - reps_lo)

    Medians over interleaved trials reject tunnel-load drift.  Returns a
    list whose min() is that time in seconds (test.py contract)."""
    assert _CACHE is not None and LAST_IN_MAPS
    runs = []
    for reps in (reps_lo, reps_hi):
        key = (reps, tuple(opts), tuple(ablate))
        if key not in _CACHE_REP:
            _CACHE_REP[key] = _build(reps, opts=opts, ablate=ablate)
        run = _get_runner(_CACHE_REP[key])
        dev = run.to_device(LAST_IN_MAPS)
        run.execute(dev)  # warm (compile + first dispatch)
        runs.append((run, dev))

    import time as _time

    # neighbor-tenant load on the shared host varies on minute timescales;
    # sample in several rounds spread over ~30s so at least one round has a
    # chance of landing in a calm window
    tlos, this = [], []
    for rd in range(rounds):
        if rd:
            _time.sleep(settle_s)
        for _ in range(trials):
            tlos.append(_launch_block(*runs[0], n_launches))
            this.append(_launch_block(*runs[1], n_launches))
    # min per leg: both minima come from the least-contended windows of the
    # same interleaved sampling period, so their difference estimates the
    # kernel's intrinsic device time free of neighbor-tenant noise (same
    # convention as a min-wall-time benchmark); medians as fallback
    tlo, thi = min(tlos), min(this)
    per_rep = (thi - tlo) / (reps_hi - reps_lo)
    if per_rep <= 0:
        tlo = sorted(tlos)[len(tlos) // 2]
        thi = sorted(this)[len(this) // 2]
        per_rep = (thi - tlo) / (reps_hi - reps_lo)
    bench.last = {"tlo": tlo, "thi": thi, "per_rep": per_rep,
                  "reps": (reps_lo, reps_hi)}
    return [per_rep]

